# revision 1
# baseline (speedup 1.0000x reference)
"""Trainium2 Bass kernel for a pre-LN decoder block (B=2, T=2048, E=1024,
H=16, HD=64, FF=4096), run SPMD across 8 NeuronCores.

Sharding: tokens are sequence-sharded. Core c (of 8) owns batch c//4 and the
interleaved token set {t : t % 4 == c % 4} (512 tokens). Interleaving makes
every core's causal workload identical, so one NEFF serves all cores (SPMD)
with per-core differences expressed purely as input data (section rotation +
mask tiles). Each core recomputes K/V for its whole batch (no collectives).

Device layouts (per core):
  x_all [2048,1024] f32   batch tokens, section-rotated so own section first
  hT    [1024,2048] bf16  LN1 output transposed (built via DMA transpose)
  qT    per head-pair [128(2h*64d), 512] bf16
  kT    per head-pair [128, 2048] bf16
  v_aug [2048, 16, 65] bf16  (65th column = 1.0 for softmax sums)
  Sexp  per (head, qtile) [128 kv, 16 chunk, 128 q] bf16
  oT    [65, 4, 128] PSUM (row 64 = softmax denominators)
"""

import math
import numpy as np
import ml_dtypes

import concourse.bass as bass
import concourse.tile as tile
from concourse import bacc, mybir
from concourse.bass_utils import run_bass_kernel_spmd

F32 = mybir.dt.float32
BF16 = mybir.dt.bfloat16
AF = mybir.ActivationFunctionType
ALU = mybir.AluOpType

B, T, E, H, HD, FF = 2, 2048, 1024, 16, 64, 4096
LN_EPS = 1e-5
SCALE = E ** -0.5          # NOTE: reference scales by E**-0.5, not HD**-0.5
NCORES = 8
TQ = 512                   # own tokens per core
NP = H // 2                # head pairs
EC = E // 128              # e-chunks
CH = T // 128              # kv chunks
QT = TQ // 128             # q tiles
FT = FF // 128             # ff tiles


def _pool(ctx_stack, tc, name, bufs, space="SBUF"):
    return ctx_stack.enter_context(tc.tile_pool(name=name, bufs=bufs, space=space))


def build_module(apply_beta1=False, apply_beta2=False, debug_taps=(), max_phase=5):
    """Build + compile the Bass module. Returns (nc, out_names)."""
    nc = bacc.Bacc("TRN2", target_bir_lowering=False, debug=False,
                   enable_asserts=True, num_devices=NCORES)

    x_all = nc.dram_tensor("x_all", [T, E], F32, kind="ExternalInput").ap()
    x_res = nc.dram_tensor("x_res", [TQ, E], F32, kind="ExternalInput").ap()
    wq = nc.dram_tensor("wq", [E, E], BF16, kind="ExternalInput").ap()
    wk = nc.dram_tensor("wk", [E, E], BF16, kind="ExternalInput").ap()
    wv = nc.dram_tensor("wv", [E, E], BF16, kind="ExternalInput").ap()
    wp = nc.dram_tensor("wp", [E, E], BF16, kind="ExternalInput").ap()
    w1 = nc.dram_tensor("w1", [E, FF], BF16, kind="ExternalInput").ap()
    w2 = nc.dram_tensor("w2", [FF, E], BF16, kind="ExternalInput").ap()
    b1c = nc.dram_tensor("b1c", [128, FT], F32, kind="ExternalInput").ap()
    be1c = nc.dram_tensor("be1c", [128, EC], F32, kind="ExternalInput").ap()
    be2c = nc.dram_tensor("be2c", [128, EC], F32, kind="ExternalInput").ap()
    maskd = nc.dram_tensor("maskd", [128, 4, 128], BF16, kind="ExternalInput").ap()
    y = nc.dram_tensor("y", [TQ, E], F32, kind="ExternalOutput").ap()

    taps = {}
    for name in debug_taps:
        shape = {
            "hT": [E, T], "qT": [E, TQ], "kT": [E, T], "vS": [T, 16 * 65],
            "sexp0": [128, CH * 128], "oTn": [E, TQ], "x2": [TQ, E],
            "h2T": [E, TQ], "gT": [FF, TQ],
        }[name]
        dt = {"x2": F32}.get(name, BF16)
        taps[name] = nc.dram_tensor("tap_" + name, shape, dt,
                                    kind="ExternalOutput").ap()

    import contextlib
    with tile.TileContext(nc) as tc, contextlib.ExitStack() as st:
        # ---------- persistent SBUF tiles ----------
        pers = _pool(st, tc, "pers", 1)
        hT = [pers.tile([128, T], BF16, tag=f"hT{ec}", name=f"hT{ec}") for ec in range(EC)]
        kT = [pers.tile([128, T], BF16, tag=f"kT{p}", name=f"kT{p}") for p in range(NP)]
        qT = [pers.tile([128, TQ], BF16, tag=f"qT{p}", name=f"qT{p}") for p in range(NP)]
        vS = [pers.tile([128, 16, 65], BF16, tag=f"vS{vt}", name=f"vS{vt}") for vt in range(CH)]
        oTn = [pers.tile([128, QT, 128], BF16, tag=f"oTn{p}", name=f"oTn{p}") for p in range(NP)]
        x2 = [pers.tile([128, E], F32, tag=f"x2{tq}", name=f"x2{tq}") for tq in range(QT)]
        h2T = [pers.tile([128, TQ], BF16, tag=f"h2T{ec}", name=f"h2T{ec}") for ec in range(EC)]
        consts = _pool(st, tc, "consts", 1)
        b1_sb = consts.tile([128, FT], F32, tag="b1", name="b1")
        mask_sb = consts.tile([128, 4, 128], BF16, tag="mask", name="mask")
        eps_sb = consts.tile([128, 1], F32, tag="eps", name="eps")
        nc.gpsimd.memset(eps_sb[:], LN_EPS)
        nc.sync.dma_start(b1_sb[:], b1c[:])
        nc.sync.dma_start(mask_sb[:], maskd[:])
        if apply_beta1:
            be1_sb = consts.tile([128, EC], F32, tag="be1", name="be1")
            nc.sync.dma_start(be1_sb[:], be1c[:])
        if apply_beta2:
            be2_sb = consts.tile([128, EC], F32, tag="be2", name="be2")
            nc.sync.dma_start(be2_sb[:], be2c[:])

        # ---------- helpers ----------
        def ln_to_transposed(src_tile, dst_tiles, dst_col0, stats_pool, scratch_pool,
                             hnat_pool):
            """LayerNorm (no affine) one [128, E] f32 tile -> bf16, then DMA-
            transpose its 8 [128,128] chunks into dst_tiles[ec][:, col0:+128]."""
            ssum = stats_pool.tile([128, 1], F32, tag="ssum", name="ssum")
            nc.vector.reduce_sum(ssum[:], src_tile[:], mybir.AxisListType.X)
            sq = scratch_pool.tile([128, E], F32, tag="sq", name="sq")
            ssq = stats_pool.tile([128, 1], F32, tag="ssq", name="ssq")
            # NOTE: tensor_tensor_reduce crashes the NRT exec unit on this
            # runtime (sim-only op here) — use mul + reduce instead.
            nc.vector.tensor_mul(sq[:], src_tile[:], src_tile[:])
            nc.vector.reduce_sum(ssq[:], sq[:], mybir.AxisListType.X)
            mu = stats_pool.tile([128, 1], F32, tag="mu", name="mu")
            nc.vector.tensor_scalar_mul(mu[:], ssum[:], 1.0 / E)
            msq = stats_pool.tile([128, 1], F32, tag="msq", name="msq")
            nc.vector.tensor_tensor(msq[:], mu[:], mu[:], ALU.mult)
            vv = stats_pool.tile([128, 1], F32, tag="vv", name="vv")
            nc.vector.tensor_scalar_mul(vv[:], ssq[:], 1.0 / E)
            nc.vector.tensor_sub(vv[:], vv[:], msq[:])
            std = stats_pool.tile([128, 1], F32, tag="std", name="std")
            nc.scalar.activation(std[:], vv[:], AF.Sqrt, bias=eps_sb[:])
            rstd = stats_pool.tile([128, 1], F32, tag="rstd", name="rstd")
            nc.vector.reciprocal(rstd[:], std[:])
            hnat = hnat_pool.tile([128, E], BF16, tag="hnat", name="hnat")
            nc.vector.tensor_scalar(hnat[:], src_tile[:], mu[:], rstd[:],
                                    ALU.subtract, ALU.mult)
            for ec in range(EC):
                nc.sync.dma_start_transpose(
                    dst_tiles[ec][:, dst_col0:dst_col0 + 128],
                    hnat[:, ec * 128:(ec + 1) * 128])

        # ---------- phase 1: LN1 + transpose ----------
        with tc.tile_pool(name="p1x", bufs=3) as p1x, \
             tc.tile_pool(name="p1s", bufs=4) as p1s, \
             tc.tile_pool(name="p1sc", bufs=2) as p1sc, \
             tc.tile_pool(name="p1h", bufs=3) as p1h:
            for tt in range(T // 128):
                xa = p1x.tile([128, E], F32, tag="xa", name="xa")
                nc.sync.dma_start(xa[:], x_all[tt * 128:(tt + 1) * 128, :])
                ln_to_transposed(xa, hT, tt * 128, p1s, p1sc, p1h)
        if apply_beta1:
            for ec in range(EC):
                nc.vector.tensor_scalar_add(hT[ec][:], hT[ec][:], be1_sb[:, ec:ec + 1])
        if "hT" in taps:
            for ec in range(EC):
                nc.sync.dma_start(taps["hT"][ec * 128:(ec + 1) * 128, :], hT[ec][:])

        # ---------- phase 2a: q & k projections (transposed layouts) ----------
        if max_phase >= 2:
            with tc.tile_pool(name="p2w", bufs=8) as p2w, \
                 tc.tile_pool(name="p2q", bufs=2, space="PSUM") as p2q, \
                 tc.tile_pool(name="p2k", bufs=6, space="PSUM") as p2k:
                for p in range(NP):
                    psq = p2q.tile([128, TQ], F32, tag="psq", name="psq")
                    for ec in range(EC):
                        wqc = p2w.tile([128, 128], BF16, tag="wqc", name="wqc")
                        nc.sync.dma_start(
                            wqc[:], wq[ec * 128:(ec + 1) * 128, p * 128:(p + 1) * 128])
                        nc.tensor.matmul(psq[:], wqc[:], hT[ec][:, 0:TQ],
                                         start=(ec == 0), stop=(ec == EC - 1))
                    nc.scalar.copy(qT[p][:], psq[:])
                    psk = [p2k.tile([128, TQ], F32, tag="psk", name="psk") for _ in range(4)]
                    for ec in range(EC):
                        wkc = p2w.tile([128, 128], BF16, tag="wkc", name="wkc")
                        nc.sync.dma_start(
                            wkc[:], wk[ec * 128:(ec + 1) * 128, p * 128:(p + 1) * 128])
                        for nt in range(4):
                            nc.tensor.matmul(psk[nt][:], wkc[:],
                                             hT[ec][:, nt * TQ:(nt + 1) * TQ],
                                             start=(ec == 0), stop=(ec == EC - 1))
                    for nt in range(4):
                        nc.vector.tensor_copy(kT[p][:, nt * TQ:(nt + 1) * TQ], psk[nt][:])

        # ---------- phase 2b: v projection (natural layout, augmented) ----------
        if max_phase >= 2:
            with tc.tile_pool(name="p2wv", bufs=1) as p2wv, \
                 tc.tile_pool(name="p2v", bufs=4, space="PSUM") as p2v:
                wv_sb = [p2wv.tile([128, E], BF16, tag=f"wv{ec}", name=f"wv{ec}") for ec in range(EC)]
                for ec in range(EC):
                    nc.sync.dma_start(wv_sb[ec][:], wv[ec * 128:(ec + 1) * 128, :])
                for vt in range(CH):
                    nc.gpsimd.memset(vS[vt][:, :, 64:65], 1.0)
                    for half in range(2):
                        psv = p2v.tile([128, 8, 64], F32, tag="psv", name="psv")
                        for ec in range(EC):
                            nc.tensor.matmul(psv[:], hT[ec][:, vt * 128:(vt + 1) * 128],
                                             wv_sb[ec][:, half * 512:(half + 1) * 512],
                                             start=(ec == 0), stop=(ec == EC - 1))
                        nc.scalar.copy(vS[vt][:, half * 8:(half + 1) * 8, 0:64], psv[:])
            if "qT" in taps:
                for p in range(NP):
                    nc.sync.dma_start(taps["qT"][p * 128:(p + 1) * 128, :], qT[p][:])
            if "kT" in taps:
                for p in range(NP):
                    nc.sync.dma_start(taps["kT"][p * 128:(p + 1) * 128, :], kT[p][:])
            if "vS" in taps:
                for vt in range(CH):
                    nc.sync.dma_start(
                        taps["vS"][vt * 128:(vt + 1) * 128, :],
                        vS[vt][:].rearrange("p h d -> p (h d)"))

        # ---------- phase 3: attention ----------
        if max_phase >= 3:
            with tc.tile_pool(name="p3strip", bufs=3, space="PSUM") as p3strip, \
                 tc.tile_pool(name="p3oT", bufs=2, space="PSUM") as p3oT, \
                 tc.tile_pool(name="p3sexp", bufs=9) as p3sexp, \
                 tc.tile_pool(name="p3sm", bufs=4) as p3sm:
                for p in range(NP):
                    sexp = {}
                    for i in range(QT):
                        # kv chunks are section-major (x_all row order: j = 4s+u).
                        # Valid set for q-tile i: all full chunks (u < i) in
                        # ascending j order, then the 4 diagonal chunks (u == i)
                        # in section order — so exp sees a contiguous slot prefix
                        # and the diagonal mask slice is always slots [4i, 4i+4).
                        valid_js = [j for j in range(CH) if j % 4 < i] + \
                                   [4 * s + i for s in range(4)]
                        nv = len(valid_js)
                        sexp[0, i] = p3sexp.tile([128, CH, 128], BF16, tag="sexp", name="sexp")
                        sexp[1, i] = p3sexp.tile([128, CH, 128], BF16, tag="sexp", name="sexp")
                        for g in range(math.ceil(nv / 8)):
                            wc = min(8, nv - 8 * g)
                            strips = [p3strip.tile([128, 8, 128], F32, tag="strip", name="strip")
                                      for _ in range(2)]
                            for jj in range(wc):
                                j = valid_js[8 * g + jj]
                                for sub in range(2):
                                    b0 = sub * 64
                                    nc.tensor.matmul(
                                        strips[sub][:, jj, :],
                                        kT[p][b0:b0 + 64, j * 128:(j + 1) * 128],
                                        qT[p][b0:b0 + 64, i * 128:(i + 1) * 128],
                                        start=True, stop=True)
                            for sub in range(2):
                                nc.scalar.activation(
                                    sexp[sub, i][:, 8 * g:8 * g + wc, :],
                                    strips[sub][:, 0:wc, :], AF.Exp, scale=SCALE)
                        for sub in range(2):
                            nc.vector.tensor_tensor(
                                sexp[sub, i][:, 4 * i:4 * i + 4, :],
                                sexp[sub, i][:, 4 * i:4 * i + 4, :],
                                mask_sb[:], ALU.mult)
                    for sub in range(2):
                        h = 2 * p + sub
                        oT = p3oT.tile([65, QT, 128], F32, tag="oT", name="oT")
                        for i in range(QT):
                            valid_js = [j for j in range(CH) if j % 4 < i] + \
                                       [4 * s + i for s in range(4)]
                            for slot, j in enumerate(valid_js):
                                nc.tensor.matmul(
                                    oT[:, i, :], vS[j][:, h, :], sexp[sub, i][:, slot, :],
                                    start=(slot == 0), stop=(slot == len(valid_js) - 1))
                        sums = p3sm.tile([1, TQ], F32, tag="sums", name="sums")
                        nc.vector.tensor_copy(sums[:], oT[64:65, :, :])
                        recip = p3sm.tile([1, TQ], F32, tag="recip", name="recip")
                        nc.vector.reciprocal(recip[:], sums[:])
                        rb = p3sm.tile([64, QT, 128], F32, tag="rb", name="rb")
                        nc.gpsimd.partition_broadcast(rb[:], recip[:])
                        nc.vector.tensor_tensor(
                            oTn[p][sub * 64:sub * 64 + 64, :, :],
                            oT[0:64, :, :], rb[:], ALU.mult)
            if "oTn" in taps:
                for p in range(NP):
                    nc.sync.dma_start(
                        taps["oTn"][p * 128:(p + 1) * 128, :],
                        oTn[p][:].rearrange("p i q -> p (i q)"))

        # ---------- phase 4: proj + residual + LN2 + transpose ----------
        if max_phase >= 4:
            with tc.tile_pool(name="p4wp", bufs=1) as p4wp, \
                 tc.tile_pool(name="p4xr", bufs=2) as p4xr, \
                 tc.tile_pool(name="p4ps", bufs=3, space="PSUM") as p4ps, \
                 tc.tile_pool(name="p4s", bufs=4) as p4s, \
                 tc.tile_pool(name="p4sc", bufs=2) as p4sc, \
                 tc.tile_pool(name="p4h", bufs=3) as p4h:
                wp_sb = [p4wp.tile([128, E], BF16, tag=f"wp{p}", name=f"wp{p}") for p in range(NP)]
                for p in range(NP):
                    nc.sync.dma_start(wp_sb[p][:], wp[p * 128:(p + 1) * 128, :])
                for tq in range(QT):
                    xr = p4xr.tile([128, E], F32, tag="xr", name="xr")
                    nc.sync.dma_start(xr[:], x_res[tq * 128:(tq + 1) * 128, :])
                    for half in range(2):
                        ps = p4ps.tile([128, 512], F32, tag="pproj", name="pproj")
                        for p in range(NP):
                            nc.tensor.matmul(ps[:], oTn[p][:, tq, :],
                                             wp_sb[p][:, half * 512:(half + 1) * 512],
                                             start=(p == 0), stop=(p == NP - 1))
                        nc.vector.tensor_add(x2[tq][:, half * 512:(half + 1) * 512],
                                             ps[:], xr[:, half * 512:(half + 1) * 512])
                    ln_to_transposed(x2[tq], h2T, tq * 128, p4s, p4sc, p4h)
                if apply_beta2:
                    for ec in range(EC):
                        nc.vector.tensor_scalar_add(h2T[ec][:], h2T[ec][:],
                                                    be2_sb[:, ec:ec + 1])
            if "x2" in taps:
                for tq in range(QT):
                    nc.sync.dma_start(taps["x2"][tq * 128:(tq + 1) * 128, :], x2[tq][:])
            if "h2T" in taps:
                for ec in range(EC):
                    nc.sync.dma_start(taps["h2T"][ec * 128:(ec + 1) * 128, :], h2T[ec][:])

        # ---------- phase 5: FFN ----------
        if max_phase >= 5:
            with tc.tile_pool(name="p5g", bufs=1) as p5g, \
                 tc.tile_pool(name="p5w", bufs=8) as p5w, \
                 tc.tile_pool(name="p5w2", bufs=4) as p5w2, \
                 tc.tile_pool(name="p5pg", bufs=3, space="PSUM") as p5pg, \
                 tc.tile_pool(name="p5py", bufs=4, space="PSUM") as p5py, \
                 tc.tile_pool(name="p5o", bufs=3) as p5o:
                gT = [p5g.tile([128, TQ], BF16, tag=f"gT{f}", name=f"gT{f}") for f in range(FT)]
                for f in range(FT):
                    ps = p5pg.tile([128, TQ], F32, tag="pg", name="pg")
                    for ec in range(EC):
                        w1c = p5w.tile([128, 128], BF16, tag="w1c", name="w1c")
                        nc.sync.dma_start(
                            w1c[:], w1[ec * 128:(ec + 1) * 128, f * 128:(f + 1) * 128])
                        nc.tensor.matmul(ps[:], w1c[:], h2T[ec][:],
                                         start=(ec == 0), stop=(ec == EC - 1))
                    nc.scalar.activation(gT[f][:], ps[:], AF.Relu, bias=b1_sb[:, f:f + 1])
                if "gT" in taps:
                    for f in range(FT):
                        nc.sync.dma_start(taps["gT"][f * 128:(f + 1) * 128, :], gT[f][:])
                for half in range(2):
                    psy = [p5py.tile([128, 512], F32, tag="py", name="py") for _ in range(QT)]
                    for f in range(FT):
                        w2c = p5w2.tile([128, 512], BF16, tag="w2c", name="w2c")
                        nc.sync.dma_start(
                            w2c[:], w2[f * 128:(f + 1) * 128, half * 512:(half + 1) * 512])
                        for tq in range(QT):
                            nc.tensor.matmul(psy[tq][:], gT[f][:, tq * 128:(tq + 1) * 128],
                                             w2c[:], start=(f == 0), stop=(f == FT - 1))
                    for tq in range(QT):
                        outsb = p5o.tile([128, 512], F32, tag="outsb", name="outsb")
                        nc.vector.tensor_add(outsb[:], psy[tq][:],
                                             x2[tq][:, half * 512:(half + 1) * 512])
                        nc.sync.dma_start(
                            y[tq * 128:(tq + 1) * 128, half * 512:(half + 1) * 512],
                            outsb[:])

    nc.compile()
    return nc


_MODULE_CACHE = {}


def _get_module(key=(False, False), debug_taps=()):
    ck = (key, tuple(debug_taps))
    if ck not in _MODULE_CACHE:
        _MODULE_CACHE[ck] = build_module(apply_beta1=key[0], apply_beta2=key[1],
                                         debug_taps=debug_taps)
    return _MODULE_CACHE[ck]


def make_core_inputs(x, Wq, Wk, Wv, Wproj, bproj, W1, b1, W2, b2, g1, be1, g2, be2):
    """Host-side sharding/folding. Returns (in_maps, meta)."""
    bf = ml_dtypes.bfloat16
    g1 = np.asarray(g1, np.float32)
    g2 = np.asarray(g2, np.float32)
    be1 = np.asarray(be1, np.float32)
    be2 = np.asarray(be2, np.float32)
    assert np.all(g1 != 0) and np.all(g2 != 0), "zero LN gamma unsupported"
    apply_beta1 = bool(np.any(be1 != 0))
    apply_beta2 = bool(np.any(be2 != 0))
    be1_eff = (be1 / g1).reshape(EC, 128).T.copy()
    be2_eff = (be2 / g2).reshape(EC, 128).T.copy()

    # lhsT layouts [E, (h, d)] with g folded into rows
    wq_l = (g1[:, None] * np.transpose(Wq, (1, 0, 2)).reshape(E, E)).astype(bf)
    wk_l = (g1[:, None] * np.transpose(Wk, (1, 0, 2)).reshape(E, E)).astype(bf)
    wv_l = (g1[:, None] * np.transpose(Wv, (1, 0, 2)).reshape(E, E)).astype(bf)
    wp_r = np.asarray(Wproj, np.float32).astype(bf)
    w1_l = (g2[:, None] * np.asarray(W1, np.float32)).astype(bf)
    w2_r = np.asarray(W2, np.float32).astype(bf)
    b1c = np.asarray(b1, np.float32).reshape(FT, 128).T.copy()

    in_maps = []
    for c in range(NCORES):
        b, own = c // 4, c % 4
        secs = [(own + s) % 4 for s in range(4)]
        x_all = np.concatenate([x[b, sig::4, :] for sig in secs], axis=0)
        x_all = np.ascontiguousarray(x_all, np.float32)
        x_resid = np.ascontiguousarray(x[b, own::4, :], np.float32) + \
            np.asarray(bproj, np.float32)[None, :]
        # mask[r, s, q] = 1 if q >= r + (sigma(s) > own)
        r = np.arange(128)[:, None, None]
        sm = np.array(secs)[None, :, None]
        q = np.arange(128)[None, None, :]
        mask = (q >= r + (sm > own)).astype(bf)
        in_maps.append({
            "x_all": x_all, "x_res": x_resid.astype(np.float32),
            "wq": wq_l, "wk": wk_l, "wv": wv_l, "wp": wp_r,
            "w1": w1_l, "w2": w2_r, "b1c": b1c,
            "be1c": be1_eff.astype(np.float32),
            "be2c": be2_eff.astype(np.float32),
            "maskd": np.ascontiguousarray(mask),
        })
    return in_maps, (apply_beta1, apply_beta2)


def assemble_output(results, b2):
    out = np.empty((B, T, E), np.float32)
    b2 = np.asarray(b2, np.float32)
    for c in range(NCORES):
        b, own = c // 4, c % 4
        out[b, own::4, :] = results[c]["y"] + b2[None, :]
    return out


def kernel(**inputs) -> np.ndarray:
    in_maps, beta_key = make_core_inputs(**inputs)
    nc = _get_module(beta_key)
    res = run_bass_kernel_spmd(nc, in_maps, core_ids=list(range(NCORES)))
    return assemble_output(res.results, inputs["b2"])

